# revision 8
# baseline (speedup 1.0000x reference)
"""Trainium2 Bass kernel v2 for single-query multi-head attention.

Reference (B=32, N=4096, D=1024, H=16, dk=dv=64):
    q = (query @ wq).reshape(B, H, dk)
    scores = einsum("bhd,bnhd->bhn", q, (key @ wk).reshape(B,N,H,dk)) / 8
    out = einsum("bhn,bnhd->bhd", softmax(scores), (value @ wv).reshape(B,N,H,dv))

Algebraic restructuring (same as v1): scores[b,:,n] = key[b,n,:] @ R_b where
R_b[:,h] = wk[:,h-block] @ q[b,h-block]; output projection by wv deferred.

v2 changes vs v1 (311 us):
  * K is pre-transposed on the host -> no on-chip K transposes (the v1 PE
    bottleneck).  R_b is precomputed on the host (q-projection prologue gone).
  * K and V stream in fp8-e3m4 (half the HBM traffic; rel_err 1.5e-2 vs the
    2e-2 gate, measured in numpy simulation of the exact quantization points).
  * scores computed transposed ([h, n], R as stationary operand, 16-col
    weight loads) with 3-way PE column tiling; small [16,128] transposes of
    the attention weights after exp.
  * softmax denominators ride the exp activation's accum_out for free; the
    final normalize + wv projection run on the host (67 MFLOP, negligible).

Sharding: data-parallel over batch, 4 batch elements per core, 8 cores.
Per-core device work is a single stream over K/V: ~32 MB fp8 -> ~90 us at the
358 GB/s per-core HBM roofline.
"""

import os
import sys

for _p in ("/opt/trn_rl_repo", os.path.expanduser("~/.axon_site/_ro/trn_rl_repo")):
    if os.path.isdir(_p) and _p not in sys.path:
        sys.path.insert(0, _p)

import numpy as np
from contextlib import ExitStack

from concourse import bass, bacc, mybir, tile, masks
from concourse.bass_utils import run_bass_kernel_spmd

N_CORES = 8
B, N, D = 32, 4096, 1024
H, DK = 16, 64
BL = B // N_CORES          # 4 batch elements per core
NT = 1024                  # key/value rows per DMA tile
NTILES = N // NT           # 4 tiles per batch
F32 = mybir.dt.float32
BF16 = mybir.dt.bfloat16
F8E3 = mybir.dt.float8e3
EXP = mybir.ActivationFunctionType.Exp

K_FP8 = os.environ.get("K_BF16") != "1" and os.environ.get("KV_BF16") != "1"
V_FP8 = os.environ.get("V_BF16") != "1" and os.environ.get("KV_BF16") != "1"
K_DT = F8E3 if K_FP8 else BF16
V_DT = F8E3 if V_FP8 else BF16


def _qslot(q):
    """(partition base, col offset) for quarter q of a [128, 512] score tile.

    matmul operands must have base_partition in {0, 32, 64}, so quarter 3
    shares column group 0 at a 256-column offset instead of using base 96.
    """
    return (32 * q, 0) if q < 3 else (0, 256)


def build_graph():
    nc = bacc.Bacc()
    r4_ext = nc.declare_dram_parameter("r4", [128, 8 * BL * H], BF16, isOutput=False)
    kt_ext = nc.declare_dram_parameter("kt", [BL, NTILES, 128, 8 * NT], K_DT,
                                       isOutput=False)
    v_ext = nc.declare_dram_parameter("v", [BL, NTILES, 128, 8 * D], V_DT,
                                      isOutput=False)
    s_ext = nc.declare_dram_parameter("s", [BL, 128, D], BF16, isOutput=True)
    z_ext = nc.declare_dram_parameter("z", [BL, 128, 2 * NTILES], F32, isOutput=True)

    with ExitStack() as ctx:
        tc = ctx.enter_context(tile.TileContext(nc))
        _body(ctx, tc, nc, r4_ext, kt_ext, v_ext, s_ext, z_ext)
    return nc


def _body(ctx, tc, nc, r4_ext, kt_ext, v_ext, s_ext, z_ext):
    const_pool = ctx.enter_context(tc.tile_pool(name="const", bufs=1))
    kt_pool = ctx.enter_context(tc.tile_pool(name="ktld", bufs=4))
    v_pool = ctx.enter_context(tc.tile_pool(name="vld", bufs=4))
    et_pool = ctx.enter_context(tc.tile_pool(name="et", bufs=3))
    es_pool = ctx.enter_context(tc.tile_pool(name="es", bufs=3))
    sums_pool = ctx.enter_context(tc.tile_pool(name="sums", bufs=2))
    sout_pool = ctx.enter_context(tc.tile_pool(name="sout", bufs=2))
    ps_sc = ctx.enter_context(tc.tile_pool(name="ps_sc", bufs=2, space="PSUM"))
    ps_tp = ctx.enter_context(tc.tile_pool(name="ps_tp", bufs=2, space="PSUM"))
    ps_va = ctx.enter_context(tc.tile_pool(name="ps_va", bufs=2, space="PSUM"))

    # constants
    r4c = const_pool.tile([128, 8 * BL * H], BF16, tag="r4c")
    nc.sync.dma_start(r4c[:], r4_ext[:])
    zb = const_pool.tile([128, 512], BF16, tag="zb")
    nc.vector.memset(zb[:], 0.0)
    identb = const_pool.tile([128, 128], BF16, tag="idb")
    masks.make_identity(nc, identb[:])

    state = {}  # pipeline state for the 1-tile software skew

    def emit_tail(b, t, st):
        """transposes + V matmuls (+ batch epilogue) for a finished tile."""
        eT, scp, v_t, vacc = st["eT"], st["scp"], st["v"], st["vacc"]
        voff = st["voff"]
        # transpose via regular matmul: out = eT_chunk.T @ selector, where
        # selector = identb[:, pb:pb+16] picks rows pb..pb+15.  Avoids PE
        # transpose-mode, which faults when interleaved with column-tiled
        # matmuls in a full engine queue.
        tp = ps_tp.tile([128, 8 * H], F32, tag="tp")
        for q in ([] if os.environ.get("NO_TP") == "1" else range(4)):
            pb, co = _qslot(q)
            for s in range(2):
                j = 2 * q + s
                nc.tensor.matmul(tp[:, j * H:(j + 1) * H],
                                 eT[:, co + s * 128:co + (s + 1) * 128],
                                 identb[:, pb:pb + 16],
                                 start=True, stop=True, skip_group_check=True)
        es = es_pool.tile([128, 8 * H], BF16, tag="es")
        for q in range(4):
            nc.vector.tensor_copy(es[:, q * 2 * H:(q + 1) * 2 * H],
                                  tp[:, q * 2 * H:(q + 1) * 2 * H])
        for dh in ([] if os.environ.get("NO_V") == "1" else range(2)):
            for sub in range(8):
                g3 = sub % 3
                last = (t == NTILES - 1) and sub >= 5
                nc.tensor.matmul(vacc[32 * g3:32 * g3 + 16, dh * 512:(dh + 1) * 512],
                                 es[:, sub * H:(sub + 1) * H],
                                 v_t[:, voff + sub * D + dh * 512:
                                      voff + sub * D + (dh + 1) * 512],
                                 start=False, stop=last,
                                 skip_group_check=True,
                                 tile_position=(0, 32 * g3))
        if t == NTILES - 1:
            s_sb = sout_pool.tile([128, D], BF16, tag="ssb")
            nc.vector.tensor_copy(s_sb[:], vacc[:])
            nc.scalar.dma_start(s_ext[b], s_sb[:])
            nc.scalar.dma_start(z_ext[b], st["sums"])

    for g in range(BL * NTILES):
        b, t = divmod(g, NTILES)
        kt_t = kt_pool.tile([128, 8 * NT], K_DT, tag="kt")
        nc.sync.dma_start(kt_t[:], kt_ext[b, t])
        v_t = v_pool.tile([128, 8 * D], V_DT, tag="v")
        v_dma = (nc.sync.dma_start if os.environ.get("V_SYNC") == "1"
                 else nc.gpsimd.dma_start)
        v_dma(v_t[:], v_ext[b, t])
        koff = 0
        voff = 0

        if t == 0:
            vacc = ps_va.tile([128, D], F32, tag="vacc", name=f"vacc{b}")
            nc.tensor.matmul(vacc[:, 0:512], zb[:, 0:128], zb[:],
                             start=True, stop=True, skip_group_check=True)
            nc.tensor.matmul(vacc[:, 512:1024], zb[:, 0:128], zb[:],
                             start=True, stop=True, skip_group_check=True)
            sums = sums_pool.tile([128, 2 * NTILES], F32, tag="sums")
            nc.vector.memset(sums[:], 0.0)
            state["vacc"], state["sums"] = vacc, sums

        # scores: scT[h, n] for 4 quarters of this tile, 3-way column tiling
        scp = ps_sc.tile([128, 512], F32, tag="scp")
        nc.tensor.matmul(scp[:], zb[:, 0:128], zb[:],
                         start=True, stop=True, skip_group_check=True)
        for dc in ([] if os.environ.get("NO_SC") == "1" else range(8)):
            for q in range(4):
                pb, co = _qslot(q)
                nc.tensor.matmul(scp[pb:pb + 16, co:co + 256],
                                 r4c[:, dc * BL * H + b * H:dc * BL * H + (b + 1) * H],
                                 kt_t[:, koff + dc * NT + q * 256:
                                      koff + dc * NT + (q + 1) * 256],
                                 start=False, stop=(dc == 7),
                                 skip_group_check=True,
                                 tile_position=(0, pb))
        # exp (+ free row-sums); memset so transpose-matmuls read no garbage
        eT = et_pool.tile([128, 512], BF16, tag="eT")
        nc.vector.memset(eT[:], 0.0)
        for q in range(4):
            pb, co = _qslot(q)
            zcol = 2 * t + co // 256
            nc.scalar.activation(eT[pb:pb + 16, co:co + 256],
                                 scp[pb:pb + 16, co:co + 256],
                                 EXP, scale=0.125,
                                 accum_out=state["sums"][pb:pb + 16, zcol:zcol + 1])

        prev = state.get("prev")
        if prev is not None:
            emit_tail(*prev)
        state["prev"] = (b, t, {"eT": eT, "scp": scp, "v": v_t, "voff": voff,
                                "vacc": state["vacc"], "sums": state["sums"]})
    emit_tail(*state["prev"])


_graph_cache = {}


def _get_graph():
    if "nc" not in _graph_cache:
        nc = build_graph()
        if not nc.is_finalized():
            nc.finalize()
        _graph_cache["nc"] = nc
    return _graph_cache["nc"]


def prepare(query, key, value, wq, wk, wv):
    """Host prework: q/R projection, K transpose, fp8 casts, per-core shards."""
    import ml_dtypes
    bf = ml_dtypes.bfloat16
    k_np = ml_dtypes.float8_e3m4 if K_FP8 else bf
    v_np = ml_dtypes.float8_e3m4 if V_FP8 else bf
    query = np.asarray(query, np.float32)
    wq = np.asarray(wq, np.float32)
    wk = np.asarray(wk, np.float32)
    q4 = query @ wq                                   # [B, H*DK]
    R = np.empty((B, D, H), np.float32)
    for h in range(H):
        R[:, :, h] = q4[:, h * DK:(h + 1) * DK] @ wk[:, h * DK:(h + 1) * DK].T
    R = R.astype(bf)
    # kt_dev[b, t, p, dc*NT + n] = key[b, t*NT + n, dc*128 + p]
    key = np.asarray(key, np.float32)
    kt_all = np.ascontiguousarray(
        key.reshape(B, NTILES, NT, 8, 128).transpose(0, 1, 4, 3, 2)
    ).astype(k_np).reshape(B, NTILES, 128, 8 * NT)
    # v_dev[b, t, p, sub*D + d] = value[b, t*NT + sub*128 + p, d]
    value = np.asarray(value, np.float32)
    v_all = np.ascontiguousarray(
        value.reshape(B, NTILES, 8, 128, D).transpose(0, 1, 3, 2, 4)
    ).astype(v_np).reshape(B, NTILES, 128, 8 * D)
    in_maps = []
    for c in range(N_CORES):
        sl = slice(c * BL, (c + 1) * BL)
        # r4c[p, dc*BL*H + f] = R[4c+ (f//H), dc*128+p, f%H]
        r4 = np.ascontiguousarray(
            R[sl].astype(np.float32).transpose(1, 0, 2)   # [D, BL, H]
            .reshape(8, 128, BL * H).transpose(1, 0, 2)   # [128, 8, BL*H]
            .reshape(128, 8 * BL * H)).astype(bf)
        in_maps.append({
            "r4": r4,
            "kt": np.ascontiguousarray(kt_all[sl]),
            "v": np.ascontiguousarray(v_all[sl]),
        })
    return in_maps


def finish(results, wv):
    """Host epilogue: combine column-group partials, normalize, project by wv."""
    wv = np.asarray(wv, np.float32)
    S = np.concatenate([np.asarray(r["s"], np.float32) for r in results], axis=0)
    Z = np.concatenate([np.asarray(r["z"], np.float32) for r in results], axis=0)
    out = np.empty((B, H * DK), np.float32)
    for h in range(H):
        s_full = S[:, h, :] + S[:, 32 + h, :] + S[:, 64 + h, :]     # [B, D]
        zf = (Z[:, h, :].sum(axis=1) + Z[:, 32 + h, 0::2].sum(axis=1)
              + Z[:, 64 + h, 0::2].sum(axis=1))                     # [B]
        out[:, h * DK:(h + 1) * DK] = (s_full / zf[:, None]) @ wv[:, h * DK:(h + 1) * DK]
    return out


LAST_RESULT = None


def kernel(query, key, value, wq, wk, wv):
    global LAST_RESULT
    nc = _get_graph()
    in_maps = prepare(query, key, value, wq, wk, wv)
    res = run_bass_kernel_spmd(nc, in_maps, core_ids=list(range(N_CORES)))
    LAST_RESULT = res
    return finish(res.results, wv)


# revision 9
# speedup vs baseline: 1.0413x; 1.0413x over previous
"""Trainium2 Bass kernel v2 for single-query multi-head attention.

Reference (B=32, N=4096, D=1024, H=16, dk=dv=64):
    q = (query @ wq).reshape(B, H, dk)
    scores = einsum("bhd,bnhd->bhn", q, (key @ wk).reshape(B,N,H,dk)) / 8
    out = einsum("bhn,bnhd->bhd", softmax(scores), (value @ wv).reshape(B,N,H,dv))

Algebraic restructuring (same as v1): scores[b,:,n] = key[b,n,:] @ R_b where
R_b[:,h] = wk[:,h-block] @ q[b,h-block]; output projection by wv deferred.

v2 changes vs v1 (311 us):
  * K is pre-transposed on the host -> no on-chip K transposes (the v1 PE
    bottleneck).  R_b is precomputed on the host (q-projection prologue gone).
  * K and V stream in fp8-e3m4 (half the HBM traffic; rel_err 1.5e-2 vs the
    2e-2 gate, measured in numpy simulation of the exact quantization points).
  * scores computed transposed ([h, n], R as stationary operand, 16-col
    weight loads) with 3-way PE column tiling; small [16,128] transposes of
    the attention weights after exp.
  * softmax denominators ride the exp activation's accum_out for free; the
    final normalize + wv projection run on the host (67 MFLOP, negligible).

Sharding: data-parallel over batch, 4 batch elements per core, 8 cores.
Per-core device work is a single stream over K/V: ~32 MB fp8 -> ~90 us at the
358 GB/s per-core HBM roofline.
"""

import os
import sys

for _p in ("/opt/trn_rl_repo", os.path.expanduser("~/.axon_site/_ro/trn_rl_repo")):
    if os.path.isdir(_p) and _p not in sys.path:
        sys.path.insert(0, _p)

import numpy as np
from contextlib import ExitStack

from concourse import bass, bacc, mybir, tile, masks
from concourse.bass_utils import run_bass_kernel_spmd

N_CORES = 8
B, N, D = 32, 4096, 1024
H, DK = 16, 64
BL = B // N_CORES          # 4 batch elements per core
NT = 1024                  # key/value rows per DMA tile
NTILES = N // NT           # 4 tiles per batch
F32 = mybir.dt.float32
BF16 = mybir.dt.bfloat16
F8E3 = mybir.dt.float8e3
EXP = mybir.ActivationFunctionType.Exp

K_FP8 = os.environ.get("K_BF16") != "1" and os.environ.get("KV_BF16") != "1"
V_FP8 = os.environ.get("V_BF16") != "1" and os.environ.get("KV_BF16") != "1"
K_DT = F8E3 if K_FP8 else BF16
V_DT = F8E3 if V_FP8 else BF16


def _qslot(q):
    """(partition base, col offset) for quarter q of a [128, 512] score tile.

    matmul operands must have base_partition in {0, 32, 64}, so quarter 3
    shares column group 0 at a 256-column offset instead of using base 96.
    """
    return (32 * q, 0) if q < 3 else (0, 256)


def build_graph():
    nc = bacc.Bacc()
    r4_ext = nc.declare_dram_parameter("r4", [128, 8 * BL * H], BF16, isOutput=False)
    kt_ext = nc.declare_dram_parameter("kt", [BL, NTILES, 128, 8 * NT], K_DT,
                                       isOutput=False)
    v_ext = nc.declare_dram_parameter("v", [BL, NTILES, 128, 8 * D], V_DT,
                                      isOutput=False)
    s_ext = nc.declare_dram_parameter("s", [BL, 128, D], F32, isOutput=True)
    z_ext = nc.declare_dram_parameter("z", [BL, 128, 2 * NTILES], F32, isOutput=True)

    with ExitStack() as ctx:
        tc = ctx.enter_context(tile.TileContext(nc))
        _body(ctx, tc, nc, r4_ext, kt_ext, v_ext, s_ext, z_ext)
    return nc


def _body(ctx, tc, nc, r4_ext, kt_ext, v_ext, s_ext, z_ext):
    const_pool = ctx.enter_context(tc.tile_pool(name="const", bufs=1))
    kt_pool = ctx.enter_context(tc.tile_pool(name="ktld", bufs=4))
    v_pool = ctx.enter_context(tc.tile_pool(name="vld", bufs=4))
    et_pool = ctx.enter_context(tc.tile_pool(name="et", bufs=3))
    es_pool = ctx.enter_context(tc.tile_pool(name="es", bufs=3))
    sums_pool = ctx.enter_context(tc.tile_pool(name="sums", bufs=2))
    sout_pool = ctx.enter_context(tc.tile_pool(name="sout", bufs=2))
    ps_sc = ctx.enter_context(tc.tile_pool(name="ps_sc", bufs=2, space="PSUM"))
    ps_tp = ctx.enter_context(tc.tile_pool(name="ps_tp", bufs=2, space="PSUM"))
    ps_va = ctx.enter_context(tc.tile_pool(name="ps_va", bufs=2, space="PSUM"))

    # constants
    r4c = const_pool.tile([128, 8 * BL * H], BF16, tag="r4c")
    nc.sync.dma_start(r4c[:], r4_ext[:])
    zb = const_pool.tile([128, 512], BF16, tag="zb")
    nc.vector.memset(zb[:], 0.0)
    identb = const_pool.tile([128, 128], BF16, tag="idb")
    masks.make_identity(nc, identb[:])

    state = {}  # pipeline state for the 1-tile software skew

    def emit_tail(b, t, st):
        """transposes + V matmuls (+ batch epilogue) for a finished tile."""
        eT, scp, v_t, vacc = st["eT"], st["scp"], st["v"], st["vacc"]
        voff = st["voff"]
        # transpose via regular matmul: out = eT_chunk.T @ selector, where
        # selector = identb[:, pb:pb+16] picks rows pb..pb+15.  Avoids PE
        # transpose-mode, which faults when interleaved with column-tiled
        # matmuls in a full engine queue.
        tp = ps_tp.tile([128, 8 * H], F32, tag="tp")
        for q in ([] if os.environ.get("NO_TP") == "1" else range(4)):
            pb, co = _qslot(q)
            for s in range(2):
                j = 2 * q + s
                nc.tensor.matmul(tp[:, j * H:(j + 1) * H],
                                 eT[:, co + s * 128:co + (s + 1) * 128],
                                 identb[:, pb:pb + 16],
                                 start=True, stop=True, skip_group_check=True)
        es = es_pool.tile([128, 8 * H], BF16, tag="es")
        nc.vector.tensor_copy(es[:], tp[:])
        for dh in ([] if os.environ.get("NO_V") == "1" else range(2)):
            for sub in range(8):
                g3 = sub % 3
                last = (t == NTILES - 1) and sub >= 5
                nc.tensor.matmul(vacc[32 * g3:32 * g3 + 16, dh * 512:(dh + 1) * 512],
                                 es[:, sub * H:(sub + 1) * H],
                                 v_t[:, voff + sub * D + dh * 512:
                                      voff + sub * D + (dh + 1) * 512],
                                 start=False, stop=last,
                                 skip_group_check=True,
                                 tile_position=(0, 32 * g3))
        if t == NTILES - 1:
            s_sb = sout_pool.tile([128, D], F32, tag="ssb")
            nc.vector.tensor_copy(s_sb[:], vacc[:])
            nc.scalar.dma_start(s_ext[b], s_sb[:])
            nc.scalar.dma_start(z_ext[b], st["sums"])

    for g in range(BL * NTILES):
        b, t = divmod(g, NTILES)
        kt_t = kt_pool.tile([128, 8 * NT], K_DT, tag="kt")
        nc.sync.dma_start(kt_t[:], kt_ext[b, t])
        v_t = v_pool.tile([128, 8 * D], V_DT, tag="v")
        v_dma = (nc.sync.dma_start if os.environ.get("V_SYNC") == "1"
                 else nc.gpsimd.dma_start)
        v_dma(v_t[:], v_ext[b, t])
        koff = 0
        voff = 0

        if t == 0:
            vacc = ps_va.tile([128, D], F32, tag="vacc", name=f"vacc{b}")
            nc.tensor.matmul(vacc[:, 0:512], zb[:, 0:128], zb[:],
                             start=True, stop=True, skip_group_check=True)
            nc.tensor.matmul(vacc[:, 512:1024], zb[:, 0:128], zb[:],
                             start=True, stop=True, skip_group_check=True)
            sums = sums_pool.tile([128, 2 * NTILES], F32, tag="sums")
            nc.vector.memset(sums[:], 0.0)
            state["vacc"], state["sums"] = vacc, sums

        # scores: scT[h, n] for 4 quarters of this tile, 3-way column tiling
        scp = ps_sc.tile([128, 512], F32, tag="scp")
        nc.tensor.matmul(scp[:], zb[:, 0:128], zb[:],
                         start=True, stop=True, skip_group_check=True)
        for dc in ([] if os.environ.get("NO_SC") == "1" else range(8)):
            for q in range(4):
                pb, co = _qslot(q)
                nc.tensor.matmul(scp[pb:pb + 16, co:co + 256],
                                 r4c[:, dc * BL * H + b * H:dc * BL * H + (b + 1) * H],
                                 kt_t[:, koff + dc * NT + q * 256:
                                      koff + dc * NT + (q + 1) * 256],
                                 start=False, stop=(dc == 7),
                                 skip_group_check=True,
                                 tile_position=(0, pb))
        # exp (+ free row-sums); memset so transpose-matmuls read no garbage
        eT = et_pool.tile([128, 512], BF16, tag="eT")
        nc.vector.memset(eT[:], 0.0)
        for q in range(4):
            pb, co = _qslot(q)
            zcol = 2 * t + co // 256
            nc.scalar.activation(eT[pb:pb + 16, co:co + 256],
                                 scp[pb:pb + 16, co:co + 256],
                                 EXP, scale=0.125,
                                 accum_out=state["sums"][pb:pb + 16, zcol:zcol + 1])

        prev = state.get("prev")
        if prev is not None:
            emit_tail(*prev)
        state["prev"] = (b, t, {"eT": eT, "scp": scp, "v": v_t, "voff": voff,
                                "vacc": state["vacc"], "sums": state["sums"]})
    emit_tail(*state["prev"])


_graph_cache = {}


def _get_graph():
    if "nc" not in _graph_cache:
        nc = build_graph()
        if not nc.is_finalized():
            nc.finalize()
        _graph_cache["nc"] = nc
    return _graph_cache["nc"]


def prepare(query, key, value, wq, wk, wv):
    """Host prework: q/R projection, K transpose, fp8 casts, per-core shards."""
    import ml_dtypes
    bf = ml_dtypes.bfloat16
    k_np = ml_dtypes.float8_e3m4 if K_FP8 else bf
    v_np = ml_dtypes.float8_e3m4 if V_FP8 else bf
    query = np.asarray(query, np.float32)
    wq = np.asarray(wq, np.float32)
    wk = np.asarray(wk, np.float32)
    q4 = query @ wq                                   # [B, H*DK]
    R = np.empty((B, D, H), np.float32)
    for h in range(H):
        R[:, :, h] = q4[:, h * DK:(h + 1) * DK] @ wk[:, h * DK:(h + 1) * DK].T
    R = R.astype(bf)
    # kt_dev[b, t, p, dc*NT + n] = key[b, t*NT + n, dc*128 + p]
    key = np.asarray(key, np.float32)
    kt_all = np.ascontiguousarray(
        key.reshape(B, NTILES, NT, 8, 128).transpose(0, 1, 4, 3, 2)
    ).astype(k_np).reshape(B, NTILES, 128, 8 * NT)
    # v_dev[b, t, p, sub*D + d] = value[b, t*NT + sub*128 + p, d]
    value = np.asarray(value, np.float32)
    v_all = np.ascontiguousarray(
        value.reshape(B, NTILES, 8, 128, D).transpose(0, 1, 3, 2, 4)
    ).astype(v_np).reshape(B, NTILES, 128, 8 * D)
    in_maps = []
    for c in range(N_CORES):
        sl = slice(c * BL, (c + 1) * BL)
        # r4c[p, dc*BL*H + f] = R[4c+ (f//H), dc*128+p, f%H]
        r4 = np.ascontiguousarray(
            R[sl].astype(np.float32).transpose(1, 0, 2)   # [D, BL, H]
            .reshape(8, 128, BL * H).transpose(1, 0, 2)   # [128, 8, BL*H]
            .reshape(128, 8 * BL * H)).astype(bf)
        in_maps.append({
            "r4": r4,
            "kt": np.ascontiguousarray(kt_all[sl]),
            "v": np.ascontiguousarray(v_all[sl]),
        })
    return in_maps


def finish(results, wv):
    """Host epilogue: combine column-group partials, normalize, project by wv."""
    wv = np.asarray(wv, np.float32)
    S = np.concatenate([np.asarray(r["s"], np.float32) for r in results], axis=0)
    Z = np.concatenate([np.asarray(r["z"], np.float32) for r in results], axis=0)
    out = np.empty((B, H * DK), np.float32)
    for h in range(H):
        s_full = S[:, h, :] + S[:, 32 + h, :] + S[:, 64 + h, :]     # [B, D]
        zf = (Z[:, h, :].sum(axis=1) + Z[:, 32 + h, 0::2].sum(axis=1)
              + Z[:, 64 + h, 0::2].sum(axis=1))                     # [B]
        out[:, h * DK:(h + 1) * DK] = (s_full / zf[:, None]) @ wv[:, h * DK:(h + 1) * DK]
    return out


LAST_RESULT = None


def kernel(query, key, value, wq, wk, wv):
    global LAST_RESULT
    nc = _get_graph()
    in_maps = prepare(query, key, value, wq, wk, wv)
    res = run_bass_kernel_spmd(nc, in_maps, core_ids=list(range(N_CORES)))
    LAST_RESULT = res
    return finish(res.results, wv)
